# revision 27
# baseline (speedup 1.0000x reference)
"""Trainium2 Bass kernel for nn_BDH_6313601925221 (sparse_attention).

Model (reference.py):
  x = LN(embed[idx])                                   (B=1, T=1024, D=256)
  repeat 6 layers (shared weights):
    x_sparse = relu(einsum('btd,hdn->bhtn', x, encoder))   N=8192, NH=4
    QR       = rope(x_sparse)                              interleaved-pair rotation
    scores   = einsum('bhtn,bhsn->bhts', QR, QR) * strict_causal
    yKV      = LN(einsum('bhts,bsd->bhtd', scores, x))
    y_sparse = relu(einsum('bhtd,hdn->bhtn', yKV, encoder_v))
    yMLP     = (x_sparse*y_sparse).transpose -> (T, NH*N) @ decoder
    x        = LN(x + LN(yMLP))
  logits = x @ lm_head

Distribution (8 cores): core c = (head h=c//2, latent-half eta=c%2).
Each core computes the encoder/rope/scores path over its 4096 latent dims
(pairwise AllReduce of partial scores within the head pair), duplicates the
small yKV path, then computes y_sparse/xy/decoder over its latent half for
all tokens; the yMLP partials are AllReduced over all 8 ranks in two
t-half chunks so the collective overlaps the remaining compute.

Key algebraic simplification: the inner LN on yKV can be dropped entirely.
LN is scale-invariant per row, relu is positively homogeneous, and the
whole yKV->y_sparse->xy->decoder path is linear in the per-token scale, so
the per-token 1/std (and the exactly-zero mean, since x rows are zero-mean
LN outputs) cancels inside the tail LN(yMLP). This lets yKV be produced
DIRECTLY in transposed [d, t] layout by the PE (lhsT = x tiles, rhs = score
strips) with no LayerNorm, no transposes.

Scheduling:
  - j-loop software-pipelined: scores matmuls for pair j-1 are emitted
    after the encoder matmuls of pair j, so the PE never waits on the
    relu->rope chain and the HAM clock stays warm.
  - rope runs on wide [128,2048] paired tiles (xs2=[xe|xo] against
    cs2=[c|s], sc2=[s|c]): 3 DVE ops + 1 GpSimd op per pair.
  - phase 3 (y_sparse/xy/decoder) runs in two 512-wide t-half passes;
    the yMLP AllReduce is split per t-half and overlaps the other pass /
    the tail. Score-strip collective bounce is 3 big DMAs per group.
  - decoder weights + lm_head resident in SBUF across all layers.

PSUM budget (8 banks): acc_a/acc_b/acc_c [128,1024] f32 (2 banks each,
bufs=1) carry scores strips, then ym accumulators (acc_a/b) and the yKV^T
accumulators (acc_c); ps_w [128,512]-padded (bufs=2) carries transient
matmul outputs.
"""

import math
import sys

import numpy as np

for _p in ("/opt/trn_rl_repo",):
    if _p not in sys.path:
        sys.path.insert(0, _p)

import concourse.bass as bass
import concourse.mybir as mybir
import concourse.tile as tile
from concourse import bacc
from concourse import bass_utils

# ---------------------------------------------------------------- constants
D = 256
NH = 4
N = 8192
T = 1024
N_LAYER = 6
VOCAB = 256
THETA = 2 ** 16
EPS = 1e-5
NCORES = 8

NHALF = N // 2          # 4096 latent dims per core
NPAIR = NHALF // 2      # 2048 rope pairs per core
NT = NHALF // 128       # 32 local n-tiles of 128
NJ = NT // 2            # 16 pair-blocks (tile 2j = evens, 2j+1 = odds)
TB = T // 128           # 8 token blocks
DC = D // 128           # 2 d-chunks

F16 = mybir.dt.float16
F32 = mybir.dt.float32
I32 = mybir.dt.int32
AX = mybir.AxisListType
ALU = mybir.AluOpType
ACTF = mybir.ActivationFunctionType

ALPHA = 1.0 / 512.0     # yKV pre-scale (overflow headroom; cancels exactly)
BETA = 1.0 / 16.0       # extra scale inside Square so sq fits fp16

# Four score-strip groups, AllReduced separately so each lands just in
# time for its consumer; the post-j-loop passes (B,C,D) supply enough PE
# work to hide every score AllReduce.  Per group: kb -> (acc tag, offset).
GROUPS = [
    {"kbs": [0, 1], "lay": {0: ("acc_a", 0), 1: ("acc_b", 0)},
     "rows": {0: 0, 1: 8}, "blocks": 15},
    {"kbs": [2], "lay": {2: ("acc_a", 0)}, "rows": {2: 0}, "blocks": 6},
    {"kbs": [3, 4], "lay": {3: ("acc_a", 0), 4: ("acc_b", 0)},
     "rows": {3: 0, 4: 5}, "blocks": 9},
    {"kbs": [5, 6, 7], "lay": {5: ("acc_a", 0), 6: ("acc_a", 512),
                               7: ("acc_b", 0)},
     "rows": {5: 0, 6: 3, 7: 5}, "blocks": 6},
]
# phase-3 / yMLP-AllReduce / tail chunks: (t-col lo, width, token blocks)
YM_CHUNKS = [(0, 512, (0, 1, 2, 3)), (512, 256, (4, 5)), (768, 256, (6, 7))]


def _ln_free(nc, pool, x_ap, eps_ap, out_f32=None, out_f16=None,
             skip_mean=False, n=None, name=""):
    """LayerNorm along the free dim of a [128, n] tile (per-partition stats)."""
    n = n if n is not None else x_ap.shape[-1]
    inv_n = 1.0 / n
    sq = pool.tile([128, n], F16, name=f"lnsq{name}", tag="lnsq")
    ssq = pool.tile([128, 1], F32, name=f"lnssq{name}", tag="lnssq")
    std = pool.tile([128, 1], F32, name=f"lnstd{name}", tag="lnstd")
    inv = pool.tile([128, 1], F32, name=f"lninv{name}", tag="lninv")
    if skip_mean:
        xm = x_ap
    else:
        mu = pool.tile([128, 1], F32, name=f"lnmu{name}", tag="lnmu")
        xm_t = pool.tile([128, n], F32, name=f"lnxm{name}", tag="lnxm")
        nc.vector.tensor_reduce(mu[:], x_ap, axis=AX.X, op=ALU.add)
        nc.scalar.mul(mu[:], mu[:], inv_n)
        nc.vector.tensor_scalar_sub(xm_t[:], x_ap, mu[:])
        xm = xm_t[:]
    nc.scalar.activation(sq[:], xm, ACTF.Square, accum_out=ssq[:])
    nc.scalar.activation(std[:], ssq[:], ACTF.Sqrt, bias=eps_ap, scale=inv_n)
    nc.vector.reciprocal(inv[:], std[:])
    if out_f32 is not None:
        nc.vector.tensor_scalar_mul(out_f32, xm, inv[:])
    if out_f16 is not None:
        nc.scalar.activation(out_f16, xm, ACTF.Copy, scale=inv[:])
    return xm, inv


def build_program(n_layer=N_LAYER):
    nc = bacc.Bacc("TRN2", target_bir_lowering=False, debug=False,
                   num_devices=NCORES)

    # ------------------------------------------------------------- I/O decl
    idx_i = nc.dram_tensor("idx32", [T, 1], F32, kind="ExternalInput")
    embed_i = nc.dram_tensor("embed", [VOCAB, D], F32, kind="ExternalInput")
    enc_i = nc.dram_tensor("enc_sh", [D, NHALF], F16, kind="ExternalInput")
    encv_i = nc.dram_tensor("encv_sh", [D, NHALF], F16, kind="ExternalInput")
    dec_i = nc.dram_tensor("dec_sh", [NHALF, D], F16, kind="ExternalInput")
    lmh_i = nc.dram_tensor("lmh", [D, VOCAB], F16, kind="ExternalInput")
    cos2_i = nc.dram_tensor("cos2_sh", [NPAIR, 2 * T], F16,
                            kind="ExternalInput")
    cmask_i = nc.dram_tensor("cmask", [128, 128], F16, kind="ExternalInput")
    out_o = nc.dram_tensor("logits", [T, VOCAB], F32, kind="ExternalOutput")

    pair_groups = [[2 * h, 2 * h + 1] for h in range(NH)]
    all_group = [list(range(NCORES))]

    with tile.TileContext(nc) as tc:
      with (
        tc.tile_pool(name="persist", bufs=1) as pp,
        tc.tile_pool(name="work", bufs=2) as wp,
        tc.tile_pool(name="psW", bufs=2, space="PSUM") as psW,
        tc.tile_pool(name="psAcc", bufs=1, space="PSUM") as psAcc,
        tc.tile_pool(name="dram", bufs=1, space="DRAM") as dp,
      ):
        # ------------------------------------------------- persistent SBUF
        enc_sb = [pp.tile([128, NHALF], F16, name=f"enc{d}", tag=f"enc{d}")
                  for d in range(DC)]
        encv_sb = [pp.tile([128, NHALF], F16, name=f"encv{d}", tag=f"encv{d}")
                   for d in range(DC)]
        dec_sb = [pp.tile([128, D], F16, name=f"dec{i}", tag=f"dec{i}")
                  for i in range(NT)]
        QR = [pp.tile([128, T], F16, name=f"qr{i}", tag=f"qr{i}")
              for i in range(NT)]
        # score strips S^T[kb]: [128 s, (TB-kb)*128 q] fp16, diag-masked
        ST = [pp.tile([128, (TB - kb) * 128], F16, name=f"st{kb}",
                      tag=f"st{kb}") for kb in range(TB)]
        x_t16 = [pp.tile([128, D], F16, name=f"xt16_{i}", tag=f"xt16_{i}")
                 for i in range(TB)]
        x_d16 = [pp.tile([128, T], F16, name=f"xd16_{i}", tag=f"xd16_{i}")
                 for i in range(DC)]
        ykvT = [pp.tile([128, T], F16, name=f"ykvT{i}", tag=f"ykvT{i}")
                for i in range(DC)]
        cmask = pp.tile([128, 128], F16, name="cmaskt", tag="cmaskt")
        eps_t = pp.tile([128, 1], F32, name="eps_t", tag="eps_t")
        ones_t = pp.tile([128, 1], F16, name="ones_t", tag="ones_t")
        eps2_t = pp.tile([1, 1], F32, name="eps2_t", tag="eps2_t")
        lmh_sb = [pp.tile([128, VOCAB], F16, name=f"lmh{d}", tag=f"lmh{d}")
                  for d in range(DC)]

        # ---------------------------------------------------- DRAM buffers
        xs_spill = dp.tile([NHALF, T], F16, name="xs_spill")
        sc_ins = [dp.tile([g["blocks"] * 128, 128], F16, name=f"sc_in{gi}",
                          tag=f"sc_in{gi}")
                  for gi, g in enumerate(GROUPS)]
        sc_outs = [dp.tile([g["blocks"] * 128, 128], F16, name=f"sc_out{gi}",
                           tag=f"sc_out{gi}")
                   for gi, g in enumerate(GROUPS)]
        ym_ins = [dp.tile([D, w], F16, name=f"ym_in{ci}", tag=f"ym_in{ci}")
                  for ci, (_, w, _) in enumerate(YM_CHUNKS)]
        ym_outs = [[dp.tile([D, w], F16, name=f"ym_out{l}_{ci}",
                            tag=f"ym_out{l}_{ci}", addr_space="Shared")
                    for ci, (_, w, _) in enumerate(YM_CHUNKS)]
                   for l in range(n_layer)]

        def psw(name, shape=(128, 512), dtype=F32):
            return psW.tile(list(shape), dtype, name=name, tag="ps_w",
                            padded_shape=[128, 512])

        # ------------------------------------------------------ load consts
        nc.gpsimd.memset(eps_t[:], EPS)
        nc.gpsimd.memset(ones_t[:], 1.0)
        nc.gpsimd.memset(eps2_t[:], EPS * ALPHA * ALPHA)
        nc.sync.dma_start(cmask[:], cmask_i[:, :])
        for d in range(DC):
            nc.sync.dma_start(enc_sb[d][:], enc_i[128 * d:128 * (d + 1), :])
            nc.sync.dma_start(encv_sb[d][:], encv_i[128 * d:128 * (d + 1), :])
            nc.sync.dma_start(lmh_sb[d][:], lmh_i[128 * d:128 * (d + 1), :])
        for i in range(NT):
            nc.scalar.dma_start(dec_sb[i][:], dec_i[128 * i:128 * (i + 1), :])

        # ------------------------------------------------------- embedding
        # E_n = LN(embed) per vocab row; x0 = onehot(idx) @ E_n
        with tc.tile_pool(name="embed", bufs=1) as ep:
            E_n = [ep.tile([128, D], F16, name=f"en{v}", tag=f"en{v}")
                   for v in range(DC)]
            for v in range(DC):
                emb_raw = ep.tile([128, D], F32, name=f"emb_raw{v}",
                                  tag=f"emb_raw{v}")
                nc.sync.dma_start(emb_raw[:], embed_i[128 * v:128 * (v + 1), :])
                _ln_free(nc, wp, emb_raw[:], eps_t[:], out_f16=E_n[v][:],
                         name=f"emb{v}")

            iota_i32 = ep.tile([128, VOCAB], I32, name="iota_i32",
                               tag="iota_i32")
            nc.gpsimd.iota(iota_i32[:], pattern=[[1, VOCAB]], base=0,
                           channel_multiplier=0)
            iota_t = ep.tile([128, VOCAB], F32, name="iota_t", tag="iota_t")
            nc.vector.tensor_copy(iota_t[:], iota_i32[:])
            OHT = [ep.tile([128, T], F16, name=f"oht{v}", tag=f"oht{v}")
                   for v in range(DC)]
            for tb in range(TB):
                idx_col = wp.tile([128, 1], F32, name="idx_col", tag="idx_col")
                nc.sync.dma_start(idx_col[:], idx_i[128 * tb:128 * (tb + 1), :])
                oh_tm = wp.tile([128, VOCAB], F16, name="oh_tm", tag="oh_tm",
                                bufs=2)
                nc.vector.tensor_scalar(oh_tm[:], iota_t[:], idx_col[:], None,
                                        op0=ALU.is_equal)
                for v in range(DC):
                    nc.sync.dma_start_transpose(
                        OHT[v][:, 128 * tb:128 * (tb + 1)],
                        oh_tm[:, 128 * v:128 * (v + 1)])

            for tb in range(TB):
                ps_x = psw(f"ps_x0_{tb}", (128, D))
                for v in range(DC):
                    nc.tensor.matmul(ps_x[:],
                                     OHT[v][:, 128 * tb:128 * (tb + 1)],
                                     E_n[v][:], start=(v == 0),
                                     stop=(v == DC - 1))
                nc.scalar.copy(x_t16[tb][:], ps_x[:])
            for d in range(DC):
                for th in range(2):
                    ps_xd = psw(f"ps_xd_{d}_{th}")
                    for v in range(DC):
                        nc.tensor.matmul(
                            ps_xd[:], E_n[v][:, 128 * d:128 * (d + 1)],
                            OHT[v][:, 512 * th:512 * (th + 1)],
                            start=(v == 0), stop=(v == DC - 1))
                    nc.scalar.copy(x_d16[d][:, 512 * th:512 * (th + 1)],
                                   ps_xd[:])

        # ============================================================ layers
        for layer in range(n_layer):
            def sc_ap(kb, lay, grp_acc):
                tag, off = lay[kb]
                w = (TB - kb) * 128
                return grp_acc[tag][:, off:off + w]

            def scores_block(nt, g, grp_acc):
                for kb in g["kbs"]:
                    dst = sc_ap(kb, g["lay"], grp_acc)
                    w = (TB - kb) * 128
                    for nn in range(0, w, 512):
                        nw = min(512, w - nn)
                        nc.tensor.matmul(
                            dst[:, nn:nn + nw],
                            QR[nt][:, 128 * kb:128 * (kb + 1)],
                            QR[nt][:, 128 * kb + nn:128 * kb + nn + nw],
                            start=(nt == 0), stop=(nt == NT - 1))

            def spill_group(gi, grp_acc):
                g = GROUPS[gi]
                for kb in g["kbs"]:
                    w = (TB - kb) * 128
                    s_sb = wp.tile([128, w], F16, name=f"s_sb{kb}",
                                   tag="s_sb", padded_shape=[128, 1024])
                    nc.scalar.copy(s_sb[:], sc_ap(kb, g["lay"], grp_acc))
                    dst = sc_ins[gi][:].rearrange("(b p) n -> p b n", p=128)
                    nc.sync.dma_start(
                        dst[:, g["rows"][kb]:g["rows"][kb] + (TB - kb), :],
                        s_sb[:].rearrange("p (b n) -> p b n", n=128))
                nc.gpsimd.collective_compute(
                    "AllReduce", ALU.add, replica_groups=pair_groups,
                    ins=[sc_ins[gi].opt()], outs=[sc_outs[gi].opt()])

            def load_groupg(gi):
                g = GROUPS[gi]
                for kb in g["kbs"]:
                    src = sc_outs[gi][:].rearrange("(b p) n -> p b n", p=128)
                    nc.sync.dma_start(
                        ST[kb][:].rearrange("p (b n) -> p b n", n=128),
                        src[:, g["rows"][kb]:g["rows"][kb] + (TB - kb), :])
                    nc.vector.tensor_mul(ST[kb][:, 0:128], ST[kb][:, 0:128],
                                         cmask[:])

            # ---------------- phase 1a: x_sparse + rope + scores group A
            accA = {t: psAcc.tile([128, 1024], F32, name=f"{t}_sA_{layer}",
                                  tag=t) for t in ("acc_a", "acc_b")}
            for j in range(NJ):
                cs2 = wp.tile([128, 2 * T], F16, name="cs2", tag="cs2",
                              bufs=2)
                nc.sync.dma_start(cs2[:], cos2_i[128 * j:128 * (j + 1), :])
                # xs2 = [xs_even | xs_odd] for this pair; t-half 0 encoder
                # matmuls first so a new layer can begin before the second
                # ym AllReduce has fully landed.
                xs2 = wp.tile([128, 2 * T], F16, name="xs2", tag="xs2",
                              bufs=3)
                for th in range(2):
                    for par in range(2):
                        nt = 2 * j + par
                        ps_e = psw(f"ps_enc_{layer}_{nt}_{th}")
                        for d in range(DC):
                            nc.tensor.matmul(
                                ps_e[:],
                                enc_sb[d][:, 128 * nt:128 * (nt + 1)],
                                x_d16[d][:, 512 * th:512 * (th + 1)],
                                start=(d == 0), stop=(d == DC - 1))
                        nc.scalar.activation(
                            xs2[:, 1024 * par + 512 * th:
                                1024 * par + 512 * (th + 1)],
                            ps_e[:], ACTF.Relu)
                nc.scalar.dma_start(
                    xs_spill[256 * j:256 * (j + 1), :].rearrange(
                        "(b p) n -> p b n", p=128),
                    xs2[:].rearrange("p (b n) -> p b n", n=T))
                # rope: cs2 = [c|s], xs2 = [xe|xo]
                #   m1 = xs2*cs2 = [xe*c | xo*s];  qe = m1_lo - m1_hi
                #   qo = xo*c + xe*s
                qe, qo = QR[2 * j], QR[2 * j + 1]
                m1 = wp.tile([128, 2 * T], F16, name="m1", tag="rope_m",
                             bufs=2)
                nc.vector.tensor_mul(m1[:], xs2[:], cs2[:])
                nc.vector.tensor_sub(qe[:], m1[:, 0:T], m1[:, T:2 * T])
                m2 = wp.tile([128, 2 * T], F16, name="m2", tag="rope_m",
                             bufs=2)
                nc.vector.tensor_mul(m2[:, 0:T], xs2[:, T:2 * T],
                                     cs2[:, 0:T])
                nc.vector.tensor_mul(m2[:, T:2 * T], xs2[:, 0:T],
                                     cs2[:, T:2 * T])
                nc.gpsimd.tensor_add(qo[:], m2[:, 0:T], m2[:, T:2 * T])
                # scores for pair j-2 (depth-2 software pipeline: PE never
                # waits on the relu/rope chain)
                if j >= 2:
                    scores_block(2 * (j - 2), GROUPS[0], accA)
                    scores_block(2 * (j - 2) + 1, GROUPS[0], accA)
            for p in (NJ - 2, NJ - 1):
                scores_block(2 * p, GROUPS[0], accA)
                scores_block(2 * p + 1, GROUPS[0], accA)
            spill_group(0, accA)

            # ---------------- phase 1b: score groups B, C, D (QR resident)
            for gi in (1, 2, 3):
                tags = sorted({GROUPS[gi]["lay"][kb][0]
                               for kb in GROUPS[gi]["kbs"]})
                accG = {t: psAcc.tile([128, 1024], F32,
                                      name=f"{t}_sg{gi}_{layer}", tag=t)
                        for t in tags}
                for nt in range(NT):
                    scores_block(nt, GROUPS[gi], accG)
                spill_group(gi, accG)

            load_groupg(0)
            load_groupg(1)
            load_groupg(2)

            # ym accumulators (tags a/b become free once C is spilled)
            ym_acc = {}
            for dh in range(DC):
                t = ("acc_a", "acc_b")[dh]
                ym_acc[dh] = psAcc.tile([128, 1024], F32,
                                        name=f"{t}_ym_{layer}", tag=t)

            def ykv_half(h):
                # ykvT[dc][:, 512h:512h+512] = sum_s x[s,dc]*ST[s, half h];
                # q-column c needs strips kb <= c//128.
                yh = psAcc.tile([128, 1024], F32, name=f"ykv_ps_{layer}_{h}",
                                tag="acc_c")
                kmax = 3 if h == 0 else TB - 1
                for dc in range(DC):
                    for kb in range(kmax + 1):
                        a = max(512 * h, 128 * kb)
                        b = 512 * (h + 1)
                        nc.tensor.matmul(
                            yh[:, 512 * dc + a - 512 * h:512 * dc + 512],
                            x_t16[kb][:, 128 * dc:128 * (dc + 1)],
                            ST[kb][:, a - 128 * kb:b - 128 * kb],
                            start=(kb == 0), stop=(kb == kmax))
                for dc in range(DC):
                    nc.scalar.mul(ykvT[dc][:, 512 * h:512 * (h + 1)],
                                  yh[:, 512 * dc:512 * dc + 512], ALPHA)

            def stats_half(h):
                # per-(head,token) 1/std of yKV over this t-half (deferred
                # inner LayerNorm; rows zero-mean so var = E[y^2]); applied
                # to the ym partials pre-AllReduce -- exact, incl. eps.
                ssq_ps = psW.tile([1, 512], F32, name=f"ssq_{layer}_{h}",
                                  tag="ps_w", padded_shape=[128, 512])
                for dc in range(DC):
                    sqt = wp.tile([128, 512], F16, name="sqt", tag="sqt",
                                  bufs=2)
                    nc.scalar.activation(sqt[:],
                                         ykvT[dc][:, 512 * h:512 * (h + 1)],
                                         ACTF.Square, scale=BETA)
                    nc.tensor.matmul(ssq_ps[:], ones_t[:], sqt[:],
                                     start=(dc == 0), stop=(dc == DC - 1))
                std_row = wp.tile([1, 512], F32, name="std_row",
                                  tag="std_row", bufs=2)
                nc.scalar.activation(std_row[:], ssq_ps[:], ACTF.Sqrt,
                                     bias=eps2_t[:],
                                     scale=1.0 / (D * BETA * BETA))
                inv_row = wp.tile([1, 512], F32, name="inv_row",
                                  tag="inv_row", bufs=2)
                nc.vector.reciprocal(inv_row[:], std_row[:])
                ib = wp.tile([128, 512], F32, name=f"inv_b{h}", tag="inv_b",
                             bufs=2)
                nc.gpsimd.partition_broadcast(ib[:], inv_row[:])
                return ib

            def emit_ym(nt, ci, xy):
                lo, w, _ = YM_CHUNKS[ci]
                for dh in range(DC):
                    nc.tensor.matmul(
                        ym_acc[dh][:, lo:lo + w],
                        dec_sb[nt][:, 128 * dh:128 * (dh + 1)],
                        xy[:], start=(nt == 0), stop=(nt == NT - 1))

            def phase3_pass(ci):
                lo, w, _ = YM_CHUNKS[ci]
                prev = None
                for nt in range(NT):
                    ps_v = psw(f"ps_ysp_{layer}_{nt}_{ci}", (128, w))
                    for d in range(DC):
                        nc.tensor.matmul(
                            ps_v[:], encv_sb[d][:, 128 * nt:128 * (nt + 1)],
                            ykvT[d][:, lo:lo + w],
                            start=(d == 0), stop=(d == DC - 1))
                    xs_c = wp.tile([128, w], F16, name="xs_c", tag="xs_c",
                                   bufs=3, padded_shape=[128, 512])
                    nc.scalar.dma_start(
                        xs_c[:],
                        xs_spill[128 * nt:128 * (nt + 1), lo:lo + w])
                    xy = wp.tile([128, w], F16, name="xy", tag="xy", bufs=3,
                                 padded_shape=[128, 512])
                    # xy = relu(ys) * xs  (fused)
                    nc.vector.scalar_tensor_tensor(
                        xy[:], ps_v[:], 0.0, xs_c[:],
                        op0=ALU.max, op1=ALU.mult)
                    if prev is not None:
                        emit_ym(*prev)
                    prev = (nt, ci, xy)
                emit_ym(*prev)

            def ym_reduce(ci, ib):
                lo, w, _ = YM_CHUNKS[ci]
                for dh in range(DC):
                    ym_sb = wp.tile([128, w], F16, name=f"ym_sb{ci}",
                                    tag="ym_sb", bufs=2,
                                    padded_shape=[128, 512])
                    nc.vector.tensor_mul(ym_sb[:], ym_acc[dh][:, lo:lo + w],
                                         ib[:, lo % 512:lo % 512 + w])
                    nc.sync.dma_start(ym_ins[ci][128 * dh:128 * (dh + 1), :],
                                      ym_sb[:])
                nc.gpsimd.collective_compute(
                    "AllReduce", ALU.add, replica_groups=all_group,
                    ins=[ym_ins[ci].opt()], outs=[ym_outs[layer][ci].opt()])

            tail_us = {}

            def tail_u(ci):
                _, _, tbs = YM_CHUNKS[ci]
                ym_out = ym_outs[layer][ci]
                for i, tb in enumerate(tbs):
                    u = wp.tile([128, D], F16, name="u_t", tag="u_t", bufs=4)
                    nc.sync.dma_start_transpose(
                        u[:], ym_out[:, 128 * i:128 * i + 128])
                    tail_us[tb] = u

            def tail_rest(ci):
                _, _, tbs = YM_CHUNKS[ci]
                for tb in tbs:
                    xm_u, inv_u = _ln_free(nc, wp, tail_us[tb][:], eps_t[:],
                                           name=f"u{tb}")
                    v = wp.tile([128, D], F32, name="v_t", tag="v_t")
                    nc.vector.scalar_tensor_tensor(
                        v[:], xm_u, inv_u[:], x_t16[tb][:],
                        op0=ALU.mult, op1=ALU.add)
                    _ln_free(nc, wp, v[:], eps_t[:],
                             out_f16=x_t16[tb][:], skip_mean=True,
                             name=f"v{tb}")
                for tb in tbs:
                    for d in range(DC):
                        nc.sync.dma_start_transpose(
                            x_d16[d][:, 128 * tb:128 * (tb + 1)],
                            x_t16[tb][:, 128 * d:128 * (d + 1)])

            # chunk 0 (t cols 0:512) needs only strips kb<=3 (groups A,B,C)
            ykv_half(0)
            ib0 = stats_half(0)
            phase3_pass(0)
            ym_reduce(0, ib0)
            tail_u(0)

            # chunks 1,2 need all strips (group D lands during chunk 0)
            load_groupg(3)
            ykv_half(1)
            ib1 = stats_half(1)
            phase3_pass(1)
            ym_reduce(1, ib1)
            # tail chunk 0 runs during chunk-2 compute; its x_d16 columns
            # unblock the next layer's t-half-0 encoder matmuls early.
            tail_rest(0)
            tail_u(1)
            phase3_pass(2)
            ym_reduce(2, ib1)
            tail_rest(1)
            tail_u(2)
            tail_rest(2)

        # ------------------------------------------------------- lm head
        for tb in range(TB):
            ps_l = psw(f"ps_lg_{tb}", (128, VOCAB))
            for d in range(DC):
                nc.tensor.matmul(ps_l[:], x_d16[d][:, 128 * tb:128 * (tb + 1)],
                                 lmh_sb[d][:], start=(d == 0),
                                 stop=(d == DC - 1))
            lg_sb = wp.tile([128, VOCAB], F32, name="lg_sb", tag="lg_sb")
            nc.vector.tensor_copy(lg_sb[:], ps_l[:])
            nc.sync.dma_start(out_o[128 * tb:128 * (tb + 1), :], lg_sb[:])

    nc.compile()
    return nc


# ------------------------------------------------------------- host helpers
def _host_tables():
    """cos/sin rope tables in [pair, t] layout, mirroring reference fp32 math."""
    n = np.arange(N, dtype=np.float32)
    q = np.floor(n / 2.0) * 2.0
    freqs = (1.0 / (np.float32(THETA) ** (q / np.float32(N)))
             / np.float32(2.0 * math.pi)).astype(np.float32)
    t = np.arange(T, dtype=np.float32)
    phases = (t[:, None] * freqs[None, :]) % 1.0
    phases = phases * np.float32(2.0 * math.pi)
    cos = np.cos(phases).astype(np.float32)   # [T, N]
    sin = np.sin(phases).astype(np.float32)
    # pair p uses freq of n=2p; table[p, t]
    cos_p = cos[:, 0::2].T.copy()  # [N//2, T]
    sin_p = sin[:, 0::2].T.copy()
    return cos_p, sin_p


def _perm_local():
    """Local latent permutation: position -> (pair index, odd flag)."""
    pos_to_pair = np.empty(NHALF, dtype=np.int64)
    pos_is_odd = np.empty(NHALF, dtype=np.int64)
    for j in range(NJ):
        pr = np.arange(128) + 128 * j
        pos_to_pair[256 * j:256 * j + 128] = pr
        pos_is_odd[256 * j:256 * j + 128] = 0
        pos_to_pair[256 * j + 128:256 * j + 256] = pr
        pos_is_odd[256 * j + 128:256 * j + 256] = 1
    return pos_to_pair, pos_is_odd


_NC_CACHE = {}


def _get_nc():
    if "nc" not in _NC_CACHE:
        _NC_CACHE["nc"] = build_program()
    return _NC_CACHE["nc"]


def prepare_in_maps(idx, embed, encoder, encoder_v, decoder, lm_head):
    idx = np.asarray(idx)
    embed = np.asarray(embed, dtype=np.float32)
    encoder = np.asarray(encoder, dtype=np.float32)
    encoder_v = np.asarray(encoder_v, dtype=np.float32)
    decoder = np.asarray(decoder, dtype=np.float32)
    lm_head = np.asarray(lm_head, dtype=np.float32)

    cos_p, sin_p = _host_tables()
    pos_to_pair, pos_is_odd = _perm_local()

    cmask = (np.arange(128)[:, None] < np.arange(128)[None, :]).astype(np.float16)
    idx32 = idx.reshape(T).astype(np.float32).reshape(T, 1)
    lmh16 = lm_head.astype(np.float16)

    in_maps = []
    for c in range(NCORES):
        h, eta = c // 2, c % 2
        pair_g = NPAIR * eta + pos_to_pair          # global pair index
        n_orig = 2 * pair_g + pos_is_odd            # original n within head
        enc_sh = encoder[h][:, n_orig].astype(np.float16)
        encv_sh = encoder_v[h][:, n_orig].astype(np.float16)
        dec_sh = decoder[h * N + n_orig, :].astype(np.float16)
        cos_sh = cos_p[NPAIR * eta:NPAIR * (eta + 1), :].astype(np.float16)
        sin_sh = sin_p[NPAIR * eta:NPAIR * (eta + 1), :].astype(np.float16)
        cos2 = np.concatenate([cos_sh, sin_sh], axis=1)  # [NPAIR, 2T] = [c|s]
        in_maps.append({
            "idx32": idx32, "embed": embed, "enc_sh": enc_sh,
            "encv_sh": encv_sh, "dec_sh": dec_sh, "lmh": lmh16,
            "cos2_sh": cos2, "cmask": cmask,
        })
    return in_maps


def kernel(idx, embed, encoder, encoder_v, decoder, lm_head):
    in_maps = prepare_in_maps(idx, embed, encoder, encoder_v, decoder,
                              lm_head)
    nc = _get_nc()
    res = bass_utils.run_bass_kernel_spmd(nc, in_maps,
                                          core_ids=list(range(NCORES)))
    _NC_CACHE["last_results"] = res
    logits = np.asarray(res.results[0]["logits"], dtype=np.float32)
    return logits.reshape(1, T, VOCAB)
